# revision 67
# baseline (speedup 1.0000x reference)
"""Bass kernel v4 for nn_Attention (channel attention / XCA block).

Per-core (one batch element, data-parallel over batch=8):
  qkv1 = w_qkv @ x; qkv = depthwise3x3(qkv1); q,k,v = split(qkv)
  q,k l2-normalized; G = q @ k^T per head; attn = softmax(G*temp)
  out = (w_proj @ blockdiag(attn)) @ v

Key structure:
- q,k path (1x1 conv + dw conv) entirely in fp8e4m3 on TensorE with
  DoubleRow (2x PE throughput; l2norm+softmax wash the quantization).
  dw = 4 DR tap-pairs on PE + center tap fused into the DVE psum-evac stt.
- pitch-130 slabs: each image row stored as [zero border, 128 cols, zero
  border], so the dx=+-1 taps read zeros at x edges -- no fixup ops.
  dw matmuls read the slab through 4D [P,2,rows,128] APs.
- v path bf16: 1x1 on PE; dw = 6 odd taps on PE diag + 3 even taps as
  DVE mul(4x)+add(2x); v accumulated into a persistent vbar.
- gram computed TRANSPOSED (lhsT=k) two strips late so its DMA-XBAR qkT
  transposes (split across SP+Act HWDGE queues) never stall PE's
  in-order stream; phase B then needs a single PE transpose per head.
- x loads dispatched from the Pool sequencer (avoids head-of-line
  blocking behind transposes on SP.SEQ); 1024-wide psum groups with one
  wide Act evac per A-tile; phase C writes bf16 output via 1024-col
  psum groups, evacs alternating Act/DVE.
"""

import sys

sys.path.insert(0, "/opt/trn_rl_repo")

import contextlib

import numpy as np
import ml_dtypes

import concourse.bass as bass
import concourse.tile as tile
from concourse import mybir
from concourse.ap import AP
from concourse.tile import add_dep_helper

BF16 = mybir.dt.bfloat16
F32 = mybir.dt.float32
F8 = mybir.dt.float8e4
NPF8 = ml_dtypes.float8_e4m3
NPBF16 = ml_dtypes.bfloat16

C = 192           # channels
OC = 3 * C        # qkv channels = 576
HEADS = 4
HC = C // HEADS   # 48
HW = 128          # image height/width
N = HW * HW       # 16384 spatial
H_STRIP = 16      # rows per strip
NSTRIP = HW // H_STRIP
OUTC = H_STRIP * 128           # dw output columns per strip
ROWP = 130                     # slab row pitch: [border, 128 cols, border]
NROWS = H_STRIP + 2            # strip rows incl. top/bottom halo
PADV = NROWS * ROWP            # slab elements per partition
OROW = 1                       # slab row holding output row y0

# all 9 dw taps: (dy, dx), slab delta = dy*ROWP + dx; border cols are
# zeroed so dx=+-1 taps read zeros at the x edges (no fixups needed).
def _tap_delta(tap):
    dy, dx = tap
    return dy * ROWP + dx

# qk dw DoubleRow tap pairs (deltas even-strided). Center (0,0) is applied
# during the psum evacuation (DVE stt with the fp8 slab).
QK_PAIR_TAPS = [((-1, -1), (-1, 1)), ((0, -1), (0, 1)), ((1, -1), (1, 1)),
                ((-1, 0), (1, 0))]
QK_PAIRS = [(_tap_delta(a), _tap_delta(b)) for a, b in QK_PAIR_TAPS]
V3_PE_TAPS = [(-1, -1), (-1, 1), (0, -1), (0, 1), (1, -1), (1, 1)]  # odd deltas
V3_DVE_TAPS = [(-1, 0), (0, 0), (1, 0)]                             # even deltas


def prep_weights(w_qkv, w_dw, temperature, w_proj):
    """Host-side weight layout prep. Returns dict of numpy arrays."""
    w_qkv = np.asarray(w_qkv, np.float32)
    w_dw = np.asarray(w_dw, np.float32).reshape(OC, 3, 3)
    w_proj = np.asarray(w_proj, np.float32)
    temperature = np.asarray(temperature, np.float32).reshape(HEADS)

    out = {}
    # qk 1x1 DoubleRow lhsT: [96, 2, 384] fp8; [p, t, m] = W1[m, t*96+p]
    w1qk = np.zeros((96, 2, 384), np.float32)
    for t in range(2):
        w1qk[:, t, :] = w_qkv[:384, t * 96:(t + 1) * 96].T
    out["w1qk8"] = np.ascontiguousarray(w1qk.reshape(96, 768)).astype(NPF8)

    # v 1x1 lhsT bf16: w1va [128, 192], w1vb [64, 192]; [k, m] = W1[384+m, k]
    w1vT = np.ascontiguousarray(w_qkv[384:].T)  # (192, 192)
    out["w1va"] = w1vT[:128].astype(NPBF16)
    out["w1vb"] = np.ascontiguousarray(w1vT[128:]).astype(NPBF16)

    # qk dw DoubleRow lhsT: per chunk [128, 4, 2, 128] fp8 diag pairs
    dwqk = np.zeros((128, 3, 4, 2, 128), np.float32)
    for i in range(3):
        for p, (tapA, tapB) in enumerate(QK_PAIR_TAPS):
            for t, tap in enumerate((tapA, tapB)):
                if tap is None:
                    continue
                dy, dx = tap
                w = w_dw[i * 128:(i + 1) * 128, 1 + dy, 1 + dx]
                np.fill_diagonal(dwqk[:, i, p, t, :], w)
    out["dwqk8"] = np.ascontiguousarray(dwqk.reshape(128, 3 * 4 * 2 * 128)).astype(NPF8)
    # qk center-tap weights (exact f32, applied at evac): [128, 3]
    out["wc_qk"] = np.ascontiguousarray(
        np.stack([w_dw[i * 128:(i + 1) * 128, 1, 1] for i in range(3)], 1),
        np.float32)

    # v chunk3 (global 384..511): PE diag bf16 for 6 odd taps
    rd3 = np.zeros((128, 6 * 128), np.float32)
    for t, (dy, dx) in enumerate(V3_PE_TAPS):
        np.fill_diagonal(rd3[:, t * 128:(t + 1) * 128],
                         w_dw[384:512, 1 + dy, 1 + dx])
    out["rdiag3"] = rd3.astype(NPBF16)
    # v chunk3 DVE taps (dy, 0): exact f32 weights
    out["wtaps3"] = np.ascontiguousarray(
        np.stack([w_dw[384:512, 1 + dy, 1] for dy in (-1, 0, 1)], 1), np.float32)

    # v chunk4 (global 512..575): 6 odd taps on PE diag bf16, 3 even on DVE
    rd4 = np.zeros((64, 6 * 64), np.float32)
    for t, (dy, dx) in enumerate(V3_PE_TAPS):
        np.fill_diagonal(rd4[:, t * 64:(t + 1) * 64],
                         w_dw[512:, 1 + dy, 1 + dx])
    out["rdiag4"] = rd4.astype(NPBF16)
    out["wtaps4"] = np.ascontiguousarray(
        np.stack([w_dw[512:, 1 + dy, 1] for dy in (-1, 0, 1)], 1), np.float32)

    # w_proj^T per head: (48, 4*192); [p, h*192+o] = w_proj[o, h*48+p]
    wpTh = np.zeros((HC, HEADS * C), np.float32)
    for h in range(HEADS):
        wpTh[:, h * C:(h + 1) * C] = w_proj[:, h * HC:(h + 1) * HC].T
    out["wpTh"] = wpTh.astype(NPBF16)

    out["temps"] = np.ascontiguousarray(
        np.broadcast_to(temperature[None, :], (HC, HEADS)), np.float32)
    out["ident48"] = np.eye(HC, dtype=np.float32)
    return out


def prep_x(x):
    """x: (B, 192, 128, 128) f32 -> per-core dicts."""
    B = x.shape[0]
    maps = []
    for b in range(B):
        xf = np.asarray(x[b], np.float32).reshape(C, N)
        x8 = np.ascontiguousarray(
            xf.reshape(2, 96, N).transpose(1, 0, 2).reshape(96, 2 * N)).astype(NPF8)
        x16 = xf.astype(NPBF16)
        maps.append({
            "x8": x8,
            "xa": np.ascontiguousarray(x16[:128]),
            "xb": np.ascontiguousarray(x16[128:]),
        })
    return maps


def build(nc):
    """Build the SPMD graph (same graph for every core)."""
    E = {}
    E["x8"] = nc.declare_dram_parameter("x8", [96, 2 * N], F8, isOutput=False)
    E["xa"] = nc.declare_dram_parameter("xa", [128, N], BF16, isOutput=False)
    E["xb"] = nc.declare_dram_parameter("xb", [64, N], BF16, isOutput=False)
    E["w1qk8"] = nc.declare_dram_parameter("w1qk8", [96, 768], F8, isOutput=False)
    E["w1va"] = nc.declare_dram_parameter("w1va", [128, 192], BF16, isOutput=False)
    E["w1vb"] = nc.declare_dram_parameter("w1vb", [64, 192], BF16, isOutput=False)
    E["dwqk8"] = nc.declare_dram_parameter("dwqk8", [128, 3 * 4 * 2 * 128], F8, isOutput=False)
    E["wc_qk"] = nc.declare_dram_parameter("wc_qk", [128, 3], F32, isOutput=False)
    E["rdiag3"] = nc.declare_dram_parameter("rdiag3", [128, 6 * 128], BF16, isOutput=False)
    E["wtaps3"] = nc.declare_dram_parameter("wtaps3", [128, 3], F32, isOutput=False)
    E["rdiag4"] = nc.declare_dram_parameter("rdiag4", [64, 6 * 64], BF16, isOutput=False)
    E["wtaps4"] = nc.declare_dram_parameter("wtaps4", [64, 3], F32, isOutput=False)
    E["wpTh"] = nc.declare_dram_parameter("wpTh", [HC, HEADS * C], BF16, isOutput=False)
    E["temps"] = nc.declare_dram_parameter("temps", [HC, HEADS], F32, isOutput=False)
    E["ident48"] = nc.declare_dram_parameter("ident48", [HC, HC], F32, isOutput=False)
    E["out"] = nc.declare_dram_parameter("out", [C, N], BF16, isOutput=True)

    terminals = []

    with tile.TileContext(nc) as tc:
        with contextlib.ExitStack() as ctx:
            _build_body(ctx, tc, nc, E, terminals)

    _split_excess_waits(nc)
    return nc


def _inst_wait_cap(inst):
    return 1


def _split_excess_waits(nc, maxw_nop=1):
    """Walrus codegen rejects instructions with >1 sem wait. Move excess
    waits onto injected same-engine NoOps placed right before the offending
    instruction."""
    n_split = 0
    for f in nc.m.functions:
        for bb in f.blocks:
            insts = bb.instructions
            out = []
            changed = False
            for inst in insts:
                si = inst.sync_info
                waits = list(si.on_wait or []) if si else []
                maxw = _inst_wait_cap(inst)
                if len(waits) > maxw:
                    keep = waits[-maxw:]
                    excess = waits[:-maxw]
                    while excess:
                        grp, excess = excess[:maxw_nop], excess[maxw_nop:]
                        n_split += 1
                        nop = mybir.InstEventSemaphore(
                            name=f"wsplit_{n_split}_{inst.name}", ins=[], outs=[])
                        nop.engine = inst.engine
                        nop.debug = inst.debug
                        nop.sync_info = mybir.SyncInfo(on_wait=grp, on_update=[])
                        nc.register_instruction(nop, overwrite=True)
                        out.append(nop)
                    si.on_wait = keep
                    changed = True
                out.append(inst)
            if changed:
                bb.instructions = out


def _pair_rows_ap(t, offset, pair_stride, nrows):
    """[P, 2, nrows, 128] DR-pair view of a pitch-ROWP slab at elem offset."""
    base = t[:]
    return AP(base.tensor, base.offset + offset,
              [list(base.ap[0]), [pair_stride, 2], [ROWP, nrows], [1, 128]])


def _rows_ap(t, offset, nrows):
    """[P, nrows, 128] view of a pitch-ROWP slab at elem offset."""
    base = t[:]
    return AP(base.tensor, base.offset + offset,
              [list(base.ap[0]), [ROWP, nrows], [1, 128]])


def _border_ap(t, col):
    """[P, NROWS, 1] view of a slab border column."""
    base = t[:]
    return AP(base.tensor, base.offset + col,
              [list(base.ap[0]), [ROWP, NROWS], [1, 1]])


def _build_body(ctx, tc, nc, E, terminals):
    AF = mybir.ActivationFunctionType
    ALU = mybir.AluOpType
    AX = mybir.AxisListType
    DR = mybir.MatmulPerfMode.DoubleRow

    singles = ctx.enter_context(tc.tile_pool(name="singles", bufs=1))
    vbar_pool = ctx.enter_context(tc.tile_pool(name="vbar", bufs=1))

    actx = contextlib.ExitStack()  # phase A+B pools; closed before phase C
    pQkv = actx.enter_context(tc.tile_pool(name="pQkv", bufs=2, space="PSUM"))
    pDw = actx.enter_context(tc.tile_pool(name="pDw", bufs=2, space="PSUM"))
    xpool = actx.enter_context(tc.tile_pool(name="xpool", bufs=2))
    slab_pool = actx.enter_context(tc.tile_pool(name="slabs", bufs=2))
    qk_pool = actx.enter_context(tc.tile_pool(name="qk", bufs=2))
    qkT_pool = actx.enter_context(tc.tile_pool(name="qkT", bufs=3))
    vmul_pool = actx.enter_context(tc.tile_pool(name="vmul", bufs=2))
    small = actx.enter_context(tc.tile_pool(name="small", bufs=2))
    pG = actx.enter_context(tc.tile_pool(name="pG", bufs=1, space="PSUM"))
    pT = actx.enter_context(tc.tile_pool(name="pT", bufs=1, space="PSUM"))

    # static evacuation engine split: Act takes 1x1 evacs (+squares),
    # DVE takes dw evacs (+v stt chains); phC alternates.
    def evac_act(dst, src):
        return nc.scalar.copy(out=dst, in_=src)

    def evac_dve(dst, src):
        return nc.vector.tensor_copy(out=dst, in_=src)

    # ---- constants ----
    w1qk8 = singles.tile([96, 2, 384], F8)
    nc.sync.dma_start(out=w1qk8[:], in_=E["w1qk8"][:].rearrange("p (t m) -> p t m", t=2))
    w1va = singles.tile([128, 192], BF16)
    nc.sync.dma_start(out=w1va[:], in_=E["w1va"][:])
    w1vb = singles.tile([64, 192], BF16)
    nc.sync.dma_start(out=w1vb[:], in_=E["w1vb"][:])
    dwqk8 = singles.tile([128, 3, 4, 2, 128], F8)
    nc.sync.dma_start(out=dwqk8[:], in_=E["dwqk8"][:].rearrange(
        "p (i q t m) -> p i q t m", i=3, q=4, t=2))
    wc_qk = singles.tile([128, 3], F32)
    nc.sync.dma_start(out=wc_qk[:], in_=E["wc_qk"][:])
    rdiag3 = singles.tile([128, 6 * 128], BF16)
    nc.sync.dma_start(out=rdiag3[:], in_=E["rdiag3"][:])
    wtaps3 = singles.tile([128, 3], F32)
    nc.sync.dma_start(out=wtaps3[:], in_=E["wtaps3"][:])
    rdiag4 = singles.tile([64, 6 * 64], BF16)
    nc.sync.dma_start(out=rdiag4[:], in_=E["rdiag4"][:])
    wtaps4 = singles.tile([64, 3], F32)
    nc.sync.dma_start(out=wtaps4[:], in_=E["wtaps4"][:])
    wpTh = singles.tile([HC, HEADS * C], BF16)
    nc.sync.dma_start(out=wpTh[:], in_=E["wpTh"][:])
    temps = singles.tile([HC, HEADS], F32)
    nc.sync.dma_start(out=temps[:], in_=E["temps"][:])
    ident48 = singles.tile([HC, HC], F32)
    i_id = nc.sync.dma_start(out=ident48[:], in_=E["ident48"][:])
    terminals.append(i_id)

    vbar_a = vbar_pool.tile([128, N], BF16)
    vbar_b = vbar_pool.tile([64, N], BF16)
    nsq_all = singles.tile([128, 3, NSTRIP], F32, name="nsq_all", tag="nsq_all")

    # G accumulator: [96, 2, 96] f32, head-pairs packed, lives all of phase A
    G = pG.tile([96, 2, HEADS // 2 * HC], F32)

    last_pe = last_act = last_dve = last_pool = None

    # per-strip gram work emitted TWO strips late so its qkT transposes
    # have ~a full strip of slack before PE's in-order stream needs them
    pend_gram = None
    pend_gram_q = []

    # gram computed TRANSPOSED (G'[d, c] = sum_n k[d] q[c]) so phase B needs
    # only one transpose per head: scale by rk on partitions, transpose,
    # then scale by rq*temp during the psum evac.
    def emit_gram(qkT, s):
        nonlocal last_pe
        for r in range(H_STRIP):
            first = (s == 0 and r == 0)
            last = (s == NSTRIP - 1 and r == H_STRIP - 1)
            for hp in range(2):
                last_pe = nc.tensor.matmul(
                    G[:, hp, :], qkT[:, r, C + hp * 96:C + (hp + 1) * 96],
                    qkT[:, r, hp * 96:(hp + 1) * 96],
                    start=first, stop=last, skip_group_check=True)

    # ---------------- phase A: strips ----------------
    for s in range(NSTRIP):
        y0 = s * H_STRIP
        ytop = max(y0 - 1, 0)
        ybot = min(y0 + H_STRIP + 1, HW)  # exclusive
        # halo reuse: strips s>0 copy rows y0-1,y0 from the previous strip's
        # slab bottom; only rows y0+1..ybot-1 are computed fresh.
        if s == 0:
            cbase_x = 0                   # first computed x column
            wrow = 1                      # slab write row
        else:
            cbase_x = (y0 + 1) * 128
            wrow = 2
        cols = ybot * 128 - cbase_x       # computed cols this strip

        # x loads first and dispatched from the (idle) Pool sequencer so they
        # are neither head-of-line blocked behind the previous strip's qkT
        # transposes (SP.SEQ) nor behind this strip's halo copies (Pool.SEQ).
        x8_t = xpool.tile([96, 2, (H_STRIP + 1) * 128], F8, tag="x8")
        xa_t = xpool.tile([128, (H_STRIP + 1) * 128], BF16, tag="xa")
        xb_t = xpool.tile([64, (H_STRIP + 1) * 128], BF16, tag="xb")
        nc.gpsimd.dma_start(out=x8_t[:, :, :cols], in_=E["x8"][:].rearrange(
            "p (t n) -> p t n", t=2)[:, :, cbase_x:ybot * 128])
        nc.gpsimd.dma_start(out=xa_t[:, :cols], in_=E["xa"][:, cbase_x:ybot * 128])
        nc.gpsimd.dma_start(out=xb_t[:, :cols], in_=E["xb"][:, cbase_x:ybot * 128])

        prev_slabs = None if s == 0 else (slab8, slabv3, slabv4)
        slab8 = [slab_pool.tile([128, PADV], F8, name=f"slab8_{i}", tag=f"slab8_{i}")
                 for i in range(3)]
        slabv3 = slab_pool.tile([128, PADV], BF16, name="slabv3", tag="slabv3")
        slabv4 = slab_pool.tile([64, PADV], BF16, name="slabv4", tag="slabv4")
        if s < 2:
            # zero the border columns once per slab buffer (never rewritten)
            for t_ in slab8 + [slabv3, slabv4]:
                nc.vector.memset(_border_ap(t_, 0), 0.0)
                nc.vector.memset(_border_ap(t_, ROWP - 1), 0.0)
        if s == 0:
            for t_ in slab8 + [slabv3, slabv4]:
                nc.vector.memset(t_[:, 1:129], 0.0)  # top halo row
        else:
            po8, pov3, pov4 = prev_slabs
            for dst, srcp in zip(slab8 + [slabv3, slabv4],
                                 po8 + [pov3, pov4]):
                last_pool = nc.gpsimd.tensor_copy(
                    out=dst[:, 0:2 * ROWP],
                    in_=srcp[:, 16 * ROWP:18 * ROWP])

        if s == NSTRIP - 1:
            for t_ in slab8 + [slabv3, slabv4]:
                nc.vector.memset(t_[:, 17 * ROWP + 1:17 * ROWP + 129], 0.0)

        qk_sb = [qk_pool.tile([128, OUTC], BF16, name=f"qk{i}", tag=f"qk{i}")
                 for i in range(3)]

        # ---- tile emitters: A-tiles drain on Act (1024-wide), B on DVE ----
        ATW = 1024  # A-tile width: 2 psum banks, one wide evac
        ntiles = (cols + ATW - 1) // ATW

        def emit_a(ci, t):
            nonlocal last_act
            base = t * ATW
            w = min(ATW, cols - base)
            if w <= 0:
                return
            row0 = wrow + base // 128
            nrows = w // 128
            if ci < 3:
                ps = pQkv.tile([128, ATW], F32, tag="pqkv", name="psa")
                for j in range(0, w, 512):
                    wj = min(512, w - j)
                    nc.tensor.matmul(
                        ps[:, j:j + wj], w1qk8[:, :, ci * 128:(ci + 1) * 128],
                        x8_t[:, :, base + j:base + j + wj],
                        start=True, stop=True, perf_mode=DR)
                last_act = evac_act(
                    _rows_ap(slab8[ci], row0 * ROWP + 1, nrows),
                    ps[:, :w].rearrange("p (r x) -> p r x", x=128))
            else:
                mb_, msz, slab = [(0, 128, slabv3), (128, 64, slabv4)][ci - 3]
                ps = pQkv.tile([msz, ATW], F32, tag="pqkv", name="psv")
                for j in range(0, w, 512):
                    wj = min(512, w - j)
                    nc.tensor.matmul(ps[:, j:j + wj], w1va[:, mb_:mb_ + msz],
                                     xa_t[:, base + j:base + j + wj],
                                     start=True, stop=False)
                    nc.tensor.matmul(ps[:, j:j + wj], w1vb[:, mb_:mb_ + msz],
                                     xb_t[:, base + j:base + j + wj],
                                     start=False, stop=True)
                last_act = evac_act(
                    _rows_ap(slab, row0 * ROWP + 1, nrows),
                    ps[:, :w].rearrange("p (r x) -> p r x", x=128))

        def emit_b(bi, nt):
            nonlocal last_act, last_dve, last_pe
            orow = OROW + nt * 4  # first output row's slab row
            obase = orow * ROWP + 1
            if bi < 3:  # qk dw chunk bi
                i = bi
                ps = pDw.tile([128, 512], F32, tag="pdw", name="psd")
                for p, (dA, dB) in enumerate(QK_PAIRS):
                    rhs = _pair_rows_ap(slab8[i], obase + dA, dB - dA, 4)
                    nc.tensor.matmul(ps[:], dwqk8[:, i, p, :, :], rhs,
                                     start=(p == 0), stop=(p == 3),
                                     perf_mode=DR)
                dst = qk_sb[i][:, nt * 512:(nt + 1) * 512]
                # evac + center tap: dst = w_c * slab8_center + psum (DVE stt)
                last_dve = nc.vector.scalar_tensor_tensor(
                    out=dst.rearrange("p (r x) -> p r x", x=128),
                    in0=_rows_ap(slab8[i], obase, 4),
                    scalar=wc_qk[:, i:i + 1],
                    in1=ps[:].rearrange("p (r x) -> p r x", x=128),
                    op0=ALU.mult, op1=ALU.add)
            else:  # v dw odd taps: bi==3 -> chunk3, bi==4 -> chunk4
                vb = vbar_a if bi == 3 else vbar_b
                rd = rdiag3 if bi == 3 else rdiag4
                csz = 128 if bi == 3 else 64
                slab = slabv3 if bi == 3 else slabv4
                ps = pDw.tile([csz, 512], F32, tag="pdw", name="psd")
                for t, tap in enumerate(V3_PE_TAPS):
                    last_pe = nc.tensor.matmul(
                        ps[:], rd[:, t * csz:(t + 1) * csz],
                        _rows_ap(slab, obase + _tap_delta(tap), 4),
                        start=(t == 0), stop=(t == 5))
                dst = vb[:, y0 * 128 + nt * 512: y0 * 128 + (nt + 1) * 512]
                last_act = evac_act(dst, ps[:])

        # ---- zipper: interleave A (1x1) and B (dw) tile emission ----
        A = [(ci, t) for ci in range(5) for t in range(ntiles)]
        B = [(bi, nt) for bi in range(5) for nt in range(OUTC // 512)]
        DELAY = 3
        ai = bi_ = 0
        while ai < len(A) or bi_ < len(B):
            if ai < len(A):
                emit_a(*A[ai]); ai += 1
            if ai >= DELAY or ai >= len(A):
                if bi_ < len(B) and (bi_ < (ai - DELAY) + 1 or ai >= len(A)):
                    emit_b(*B[bi_]); bi_ += 1
        # emit the oldest pending gram (two strips back) after this strip's
        # dw work so PE (in-order) does not head-of-line block on its
        # transposes.
        if pend_gram_q:
            emit_gram(*pend_gram_q.pop(0))

        # --- norms: sum of squares per channel on Act ---
        sq_scr = vmul_pool.tile([128, OUTC], BF16, tag="vp", name="sq_scr")
        for i in range(3):
            last_act = nc.scalar.activation(
                out=sq_scr[:], in_=qk_sb[i][:], func=AF.Square,
                accum_out=nsq_all[:, i, s:s + 1])

        # --- qk transpose via DMA XBAR into [x, r, ch]; split across the
        # two HWDGE queues (SP, Act) so the transposes overlap ---
        qkT = qkT_pool.tile([128, H_STRIP, 2 * C], BF16, tag="qkT")
        nc.sync.dma_start_transpose(out=qkT[:, :, 0:128], in_=qk_sb[0][:])
        nc.scalar.dma_start_transpose(out=qkT[:, :, 128:256], in_=qk_sb[1][:])
        nc.sync.dma_start_transpose(out=qkT[:, 0:8, 256:384],
                                    in_=qk_sb[2][:, 0:1024])
        nc.scalar.dma_start_transpose(out=qkT[:, 8:16, 256:384],
                                      in_=qk_sb[2][:, 1024:2048])
        if pend_gram is not None:
            pend_gram_q.append(pend_gram)
        pend_gram = (qkT, s)

        # --- v even taps: mul (4x) into scratch, add (2x) in-place to vbar ---
        va_sl = vbar_a[:, y0 * 128: y0 * 128 + OUTC]
        vb_sl = vbar_b[:, y0 * 128: y0 * 128 + OUTC]
        for ci, (slab, wt, sl, csz) in enumerate(
                [(slabv3, wtaps3, va_sl, 128), (slabv4, wtaps4, vb_sl, 64)]):
            for j, dy in enumerate((-1, 0, 1)):
                o = (OROW + dy) * ROWP + 1
                pr = vmul_pool.tile([csz, OUTC], BF16, tag="vp", name="pr")
                last_dve = nc.vector.tensor_scalar_mul(
                    pr[:].rearrange("p (r x) -> p r x", x=128),
                    _rows_ap(slab, o, H_STRIP), wt[:, j:j + 1])
                last_dve = nc.vector.tensor_add(sl, sl, pr[:])

    # tail grams
    for g in pend_gram_q:
        emit_gram(*g)
    pend_gram_q = []
    if pend_gram is not None:
        emit_gram(*pend_gram)
        pend_gram = None

    # ---------------- phase B ----------------
    Gsb = small.tile([96, 2, 96], F32, tag="gsb")
    last_act = nc.scalar.copy(out=Gsb[:], in_=G[:])

    # batched norm chain: one reduce/sqrt/recip over all 3 chunks
    tot = small.tile([128, 3, 1], F32, tag="tot")
    nc.vector.tensor_reduce(out=tot[:], in_=nsq_all[:], axis=AX.X, op=ALU.add)
    rt = small.tile([128, 3, 1], F32, tag="rt")
    nc.scalar.sqrt(out=rt[:], in_=tot[:])
    rr = small.tile([128, 3, 1], F32, tag="rr")
    nc.vector.reciprocal(out=rr[:], in_=rt[:])

    def dmaq(**kw):
        return nc.sync.dma_start(**kw)

    def gather_head(dst, global_base):
        done = 0
        g = global_base
        while done < HC:
            oc, off = g // 128, g % 128
            take = min(HC - done, 128 - off)
            dmaq(out=dst[done:done + take, :],
                 in_=rr[off:off + take, oc, 0:1])
            done += take
            g += take

    mh_sb = []
    for h in range(HEADS):
        hp, off = h // 2, (h % 2) * HC
        rq = small.tile([HC, 1], F32, tag="rq")
        gather_head(rq, h * HC)
        rk = small.tile([HC, 1], F32, tag="rk")
        gather_head(rk, C + h * HC)
        rqt = small.tile([HC, 1], F32, tag="rqt")
        nc.vector.tensor_mul(rqt[:], rq[:], temps[:, h:h + 1])
        g_h = small.tile([HC, HC], F32, tag="gh")
        nc.gpsimd.dma_start(out=g_h[:], in_=Gsb[off:off + HC, hp, off:off + HC])
        # g_h is G'[d, c]; scale rows by rk, transpose, then exp with the
        # rq*temp scale fused in (|z| <= temp, no max-shift needed). The
        # softmax 1/sum normalization commutes through the mh matmul as a
        # per-row scale, applied at the msb evac -- so rs is off the
        # critical path.
        z1 = small.tile([HC, HC], F32, tag="z1")
        nc.vector.tensor_scalar_mul(z1[:], g_h[:], rk[:])
        z1T_ps = pT.tile([HC, HC], F32, tag="ptz")
        nc.tensor.transpose(z1T_ps[:], z1[:], ident48[:])
        e = small.tile([HC, HC], F32, tag="e")
        ssum = small.tile([HC, 1], F32, tag="ssum")
        nc.scalar.activation(out=e[:], in_=z1T_ps[:], func=AF.Exp,
                             scale=rqt[:], accum_out=ssum[:])
        rs = small.tile([HC, 1], F32, tag="rs")
        nc.vector.reciprocal(rs[:], ssum[:])
        attn = small.tile([HC, HC], BF16, tag="attn")
        last_dve = nc.vector.tensor_copy(out=attn[:], in_=e[:])
        mh = pDw.tile([HC, C], F32, tag="pdw")
        nc.tensor.matmul(mh[:], attn[:], wpTh[:, h * C:(h + 1) * C],
                         start=True, stop=True)
        msb = small.tile([HC, C], BF16, tag=f"msb{h}")
        nc.scalar.activation(out=msb[:], in_=mh[:], func=AF.Copy, scale=rs[:])
        mh_sb.append(msb)

    MTa = singles.tile([128, C], BF16)
    MTb = singles.tile([64, C], BF16)
    dmaq(out=MTa[0:48, :], in_=mh_sb[0][:])
    dmaq(out=MTa[48:96, :], in_=mh_sb[1][:])
    dmaq(out=MTa[96:128, :], in_=mh_sb[2][0:32, :])
    dmaq(out=MTb[0:16, :], in_=mh_sb[2][32:48, :])
    i_m = nc.sync.dma_start(out=MTb[16:64, :], in_=mh_sb[3][:])
    terminals.append(i_m)

    # ---------------- phase C: out = blockdiag-attn-proj @ vbar ----------------
    actx.close()  # free phase-A SBUF for wide output staging
    outp = ctx.enter_context(tc.tile_pool(name="outp", bufs=3))
    pC0 = ctx.enter_context(tc.tile_pool(name="pC0", bufs=2, space="PSUM"))
    pC1 = ctx.enter_context(tc.tile_pool(name="pC1", bufs=2, space="PSUM"))
    BLK = 1024
    for blk in range(N // BLK):
        ps0 = pC0.tile([128, BLK], F32, tag="pc0")
        ps1 = pC1.tile([64, BLK], F32, tag="pc1")
        for j in range(BLK // 512):
            sl = slice(blk * BLK + j * 512, blk * BLK + (j + 1) * 512)
            jsl = slice(j * 512, (j + 1) * 512)
            nc.tensor.matmul(ps0[:, jsl], MTa[:, 0:128], vbar_a[:, sl], start=True, stop=False)
            nc.tensor.matmul(ps0[:, jsl], MTb[:, 0:128], vbar_b[:, sl], start=False, stop=True)
            nc.tensor.matmul(ps1[:, jsl], MTa[:, 128:192], vbar_a[:, sl], start=True, stop=False)
            last_pe = nc.tensor.matmul(ps1[:, jsl], MTb[:, 128:192], vbar_b[:, sl],
                                       start=False, stop=True)
        o0 = outp.tile([128, BLK], BF16, tag="o0")
        o1 = outp.tile([64, BLK], BF16, tag="o1")
        if blk % 2 == 0:
            last_act = nc.scalar.copy(out=o0[:], in_=ps0[:])
            last_dve = nc.vector.tensor_copy(out=o1[:], in_=ps1[:])
        else:
            last_dve = nc.vector.tensor_copy(out=o0[:], in_=ps0[:])
            last_act = nc.scalar.copy(out=o1[:], in_=ps1[:])
        i0 = nc.sync.dma_start(out=E["out"][0:128, blk * BLK:(blk + 1) * BLK], in_=o0[:])
        i1 = nc.gpsimd.dma_start(out=E["out"][128:192, blk * BLK:(blk + 1) * BLK], in_=o1[:])
        terminals.append(i0)
        terminals.append(i1)

    terminals.append(last_pe)
    terminals.append(last_act)
    terminals.append(last_dve)
    terminals.append(last_pool)


# ----------------------------------------------------------------------------
# Public entry point: full inputs -> full output, 8-way data-parallel over
# batch across NeuronCores 0-7.
# ----------------------------------------------------------------------------

def kernel(x, w_qkv, w_dw, temperature, w_proj):
    from concourse.bass_utils import run_bass_kernel_spmd

    x = np.asarray(x, np.float32)
    B = x.shape[0]
    assert x.shape == (8, C, HW, HW), x.shape

    nc = bass.Bass()
    build(nc)

    wmaps = prep_weights(w_qkv, w_dw, temperature, w_proj)
    xmaps = prep_x(x)
    in_maps = [{**wmaps, **xm} for xm in xmaps]

    res = run_bass_kernel_spmd(nc, in_maps, core_ids=list(range(8)))
    out = np.stack([np.asarray(res.results[b]["out"]).astype(np.float32)
                    .reshape(C, HW, HW) for b in range(B)])
    return out



# revision 79
# speedup vs baseline: 1.2102x; 1.2102x over previous
"""Bass kernel v4 for nn_Attention (channel attention / XCA block).

Per-core (one batch element, data-parallel over batch=8):
  qkv1 = w_qkv @ x; qkv = depthwise3x3(qkv1); q,k,v = split(qkv)
  q,k l2-normalized; G = q @ k^T per head; attn = softmax(G*temp)
  out = (w_proj @ blockdiag(attn)) @ v

Key structure:
- q,k path (1x1 conv + dw conv) entirely in fp8e4m3 on TensorE with
  DoubleRow (2x PE throughput; l2norm+softmax wash the quantization).
  dw = 4 DR tap-pairs on PE + center tap fused into the DVE psum-evac stt.
- pitch-130 slabs: each image row stored as [zero border, 128 cols, zero
  border], so the dx=+-1 taps read zeros at x edges -- no fixup ops.
  dw matmuls read the slab through 4D [P,2,rows,128] APs.
- v path bf16: 1x1 on PE; dw = 6 odd taps on PE diag + 3 even taps as
  DVE mul(4x)+add(2x); v accumulated into a persistent vbar.
- gram computed TRANSPOSED (lhsT=k) two strips late so its DMA-XBAR qkT
  transposes (split across SP+Act HWDGE queues) never stall PE's
  in-order stream; phase B then needs a single PE transpose per head.
- x loads dispatched from the Pool sequencer (avoids head-of-line
  blocking behind transposes on SP.SEQ); 1024-wide psum groups with one
  wide Act evac per A-tile; phase C writes bf16 output via 1024-col
  psum groups, evacs alternating Act/DVE.
"""

import sys

sys.path.insert(0, "/opt/trn_rl_repo")

import contextlib

import numpy as np
import ml_dtypes

import concourse.bass as bass
import concourse.tile as tile
from concourse import mybir
from concourse.ap import AP
from concourse.tile import add_dep_helper

BF16 = mybir.dt.bfloat16
F32 = mybir.dt.float32
F8 = mybir.dt.float8e4
NPF8 = ml_dtypes.float8_e4m3
NPBF16 = ml_dtypes.bfloat16

C = 192           # channels
OC = 3 * C        # qkv channels = 576
HEADS = 4
HC = C // HEADS   # 48
HW = 128          # image height/width
N = HW * HW       # 16384 spatial
H_STRIP = 16      # rows per strip
NSTRIP = HW // H_STRIP
OUTC = H_STRIP * 128           # dw output columns per strip
ROWP = 130                     # slab row pitch: [border, 128 cols, border]
NROWS = H_STRIP + 2            # strip rows incl. top/bottom halo
PADV = NROWS * ROWP            # slab elements per partition
OROW = 1                       # slab row holding output row y0

# all 9 dw taps: (dy, dx), slab delta = dy*ROWP + dx; border cols are
# zeroed so dx=+-1 taps read zeros at the x edges (no fixups needed).
def _tap_delta(tap):
    dy, dx = tap
    return dy * ROWP + dx

# qk dw DoubleRow tap pairs (deltas even-strided). Center (0,0) is applied
# during the psum evacuation (DVE stt with the fp8 slab).
QK_PAIR_TAPS = [((-1, -1), (-1, 1)), ((0, -1), (0, 1)), ((1, -1), (1, 1)),
                ((-1, 0), (1, 0))]
QK_PAIRS = [(_tap_delta(a), _tap_delta(b)) for a, b in QK_PAIR_TAPS]
V3_PE_TAPS = [(-1, -1), (-1, 1), (0, -1), (0, 1), (1, -1), (1, 1)]  # odd deltas
V3_DVE_TAPS = [(-1, 0), (0, 0), (1, 0)]                             # even deltas


def prep_weights(w_qkv, w_dw, temperature, w_proj):
    """Host-side weight layout prep. Returns dict of numpy arrays."""
    w_qkv = np.asarray(w_qkv, np.float32)
    w_dw = np.asarray(w_dw, np.float32).reshape(OC, 3, 3)
    w_proj = np.asarray(w_proj, np.float32)
    temperature = np.asarray(temperature, np.float32).reshape(HEADS)

    out = {}
    # qk 1x1 DoubleRow lhsT: [96, 2, 384] fp8; [p, t, m] = W1[m, t*96+p]
    w1qk = np.zeros((96, 2, 384), np.float32)
    for t in range(2):
        w1qk[:, t, :] = w_qkv[:384, t * 96:(t + 1) * 96].T
    out["w1qk8"] = np.ascontiguousarray(w1qk.reshape(96, 768)).astype(NPF8)

    # v 1x1 lhsT bf16: w1va [128, 192], w1vb [64, 192]; [k, m] = W1[384+m, k]
    w1vT = np.ascontiguousarray(w_qkv[384:].T)  # (192, 192)
    out["w1va"] = w1vT[:128].astype(NPBF16)
    out["w1vb"] = np.ascontiguousarray(w1vT[128:]).astype(NPBF16)

    # qk dw DoubleRow lhsT: per chunk [128, 4, 2, 128] fp8 diag pairs
    dwqk = np.zeros((128, 3, 4, 2, 128), np.float32)
    for i in range(3):
        for p, (tapA, tapB) in enumerate(QK_PAIR_TAPS):
            for t, tap in enumerate((tapA, tapB)):
                if tap is None:
                    continue
                dy, dx = tap
                w = w_dw[i * 128:(i + 1) * 128, 1 + dy, 1 + dx]
                np.fill_diagonal(dwqk[:, i, p, t, :], w)
    out["dwqk8"] = np.ascontiguousarray(dwqk.reshape(128, 3 * 4 * 2 * 128)).astype(NPF8)
    # qk center-tap weights (exact f32, applied at evac): [128, 3]
    out["wc_qk"] = np.ascontiguousarray(
        np.stack([w_dw[i * 128:(i + 1) * 128, 1, 1] for i in range(3)], 1),
        np.float32)

    # v chunk3 (global 384..511): PE diag bf16 for 6 odd taps
    rd3 = np.zeros((128, 6 * 128), np.float32)
    for t, (dy, dx) in enumerate(V3_PE_TAPS):
        np.fill_diagonal(rd3[:, t * 128:(t + 1) * 128],
                         w_dw[384:512, 1 + dy, 1 + dx])
    out["rdiag3"] = rd3.astype(NPBF16)
    # v chunk3 DVE taps (dy, 0): exact f32 weights
    out["wtaps3"] = np.ascontiguousarray(
        np.stack([w_dw[384:512, 1 + dy, 1] for dy in (-1, 0, 1)], 1), np.float32)

    # v chunk4 (global 512..575): 6 odd taps on PE diag bf16, 3 even on DVE
    rd4 = np.zeros((64, 6 * 64), np.float32)
    for t, (dy, dx) in enumerate(V3_PE_TAPS):
        np.fill_diagonal(rd4[:, t * 64:(t + 1) * 64],
                         w_dw[512:, 1 + dy, 1 + dx])
    out["rdiag4"] = rd4.astype(NPBF16)
    out["wtaps4"] = np.ascontiguousarray(
        np.stack([w_dw[512:, 1 + dy, 1] for dy in (-1, 0, 1)], 1), np.float32)

    # w_proj^T per head: (48, 4*192); [p, h*192+o] = w_proj[o, h*48+p]
    wpTh = np.zeros((HC, HEADS * C), np.float32)
    for h in range(HEADS):
        wpTh[:, h * C:(h + 1) * C] = w_proj[:, h * HC:(h + 1) * HC].T
    out["wpTh"] = wpTh.astype(NPBF16)

    out["temps"] = np.ascontiguousarray(
        np.broadcast_to(temperature[None, :], (HC, HEADS)), np.float32)
    out["ident48"] = np.eye(HC, dtype=np.float32)

    # pack the 12 weight tensors into 3 dtype-blobs: the measurement rig
    # pays ~12us of per-execution binding overhead per input tensor.
    wb8 = np.zeros((128, 3840), NPF8)
    wb8[:96, 0:768] = out.pop("w1qk8")
    wb8[:, 768:3840] = out.pop("dwqk8")
    wb16 = np.zeros((128, 2304), NPBF16)
    wb16[:, 0:192] = out.pop("w1va")
    wb16[:64, 192:384] = out.pop("w1vb")
    wb16[:, 384:1152] = out.pop("rdiag3")
    wb16[:64, 1152:1536] = out.pop("rdiag4")
    wb16[:48, 1536:2304] = out.pop("wpTh")
    wbf = np.zeros((128, 61), np.float32)
    wbf[:, 0:3] = out.pop("wc_qk")
    wbf[:, 3:6] = out.pop("wtaps3")
    wbf[:64, 6:9] = out.pop("wtaps4")
    wbf[:48, 9:13] = out.pop("temps")
    wbf[:48, 13:61] = out.pop("ident48")
    out["wb8"], out["wb16"], out["wbf"] = wb8, wb16, wbf
    return out


def prep_x(x):
    """x: (B, 192, 128, 128) f32 -> per-core dicts."""
    B = x.shape[0]
    maps = []
    for b in range(B):
        xf = np.asarray(x[b], np.float32).reshape(C, N)
        x8 = np.ascontiguousarray(
            xf.reshape(2, 96, N).transpose(1, 0, 2).reshape(96, 2 * N)).astype(NPF8)
        x16 = xf.astype(NPBF16)
        maps.append({
            "x8": x8,
            "xa": np.ascontiguousarray(x16[:128]),
            "xb": np.ascontiguousarray(x16[128:]),
        })
    return maps


def build(nc):
    """Build the SPMD graph (same graph for every core)."""
    E = {}
    E["x8"] = nc.declare_dram_parameter("x8", [96, 2 * N], F8, isOutput=False)
    E["xa"] = nc.declare_dram_parameter("xa", [128, N], BF16, isOutput=False)
    E["xb"] = nc.declare_dram_parameter("xb", [64, N], BF16, isOutput=False)
    E["wb8"] = nc.declare_dram_parameter("wb8", [128, 3840], F8, isOutput=False)
    E["wb16"] = nc.declare_dram_parameter("wb16", [128, 2304], BF16, isOutput=False)
    E["wbf"] = nc.declare_dram_parameter("wbf", [128, 61], F32, isOutput=False)
    E["out"] = nc.declare_dram_parameter("out", [C, N], BF16, isOutput=True)

    terminals = []

    with tile.TileContext(nc) as tc:
        with contextlib.ExitStack() as ctx:
            _build_body(ctx, tc, nc, E, terminals)

    _split_excess_waits(nc)
    return nc


def _inst_wait_cap(inst):
    return 1


def _split_excess_waits(nc, maxw_nop=1):
    """Walrus codegen rejects instructions with >1 sem wait. Move excess
    waits onto injected same-engine NoOps placed right before the offending
    instruction."""
    n_split = 0
    for f in nc.m.functions:
        for bb in f.blocks:
            insts = bb.instructions
            out = []
            changed = False
            for inst in insts:
                si = inst.sync_info
                waits = list(si.on_wait or []) if si else []
                maxw = _inst_wait_cap(inst)
                if len(waits) > maxw:
                    keep = waits[-maxw:]
                    excess = waits[:-maxw]
                    while excess:
                        grp, excess = excess[:maxw_nop], excess[maxw_nop:]
                        n_split += 1
                        nop = mybir.InstEventSemaphore(
                            name=f"wsplit_{n_split}_{inst.name}", ins=[], outs=[])
                        nop.engine = inst.engine
                        nop.debug = inst.debug
                        nop.sync_info = mybir.SyncInfo(on_wait=grp, on_update=[])
                        nc.register_instruction(nop, overwrite=True)
                        out.append(nop)
                    si.on_wait = keep
                    changed = True
                out.append(inst)
            if changed:
                bb.instructions = out


def _pair_rows_ap(t, offset, pair_stride, nrows):
    """[P, 2, nrows, 128] DR-pair view of a pitch-ROWP slab at elem offset."""
    base = t[:]
    return AP(base.tensor, base.offset + offset,
              [list(base.ap[0]), [pair_stride, 2], [ROWP, nrows], [1, 128]])


def _rows_ap(t, offset, nrows):
    """[P, nrows, 128] view of a pitch-ROWP slab at elem offset."""
    base = t[:]
    return AP(base.tensor, base.offset + offset,
              [list(base.ap[0]), [ROWP, nrows], [1, 128]])


def _border_ap(t, col):
    """[P, NROWS, 1] view of a slab border column."""
    base = t[:]
    return AP(base.tensor, base.offset + col,
              [list(base.ap[0]), [ROWP, NROWS], [1, 1]])


def _build_body(ctx, tc, nc, E, terminals):
    AF = mybir.ActivationFunctionType
    ALU = mybir.AluOpType
    AX = mybir.AxisListType
    DR = mybir.MatmulPerfMode.DoubleRow

    singles = ctx.enter_context(tc.tile_pool(name="singles", bufs=1))
    vbar_pool = ctx.enter_context(tc.tile_pool(name="vbar", bufs=1))

    actx = contextlib.ExitStack()  # phase A+B pools; closed before phase C
    pQkv = actx.enter_context(tc.tile_pool(name="pQkv", bufs=2, space="PSUM"))
    pDw = actx.enter_context(tc.tile_pool(name="pDw", bufs=2, space="PSUM"))
    xpool = actx.enter_context(tc.tile_pool(name="xpool", bufs=2))
    slab_pool = actx.enter_context(tc.tile_pool(name="slabs", bufs=2))
    qk_pool = actx.enter_context(tc.tile_pool(name="qk", bufs=2))
    qkT_pool = actx.enter_context(tc.tile_pool(name="qkT", bufs=3))
    vmul_pool = actx.enter_context(tc.tile_pool(name="vmul", bufs=2))
    small = actx.enter_context(tc.tile_pool(name="small", bufs=2))
    pG = actx.enter_context(tc.tile_pool(name="pG", bufs=1, space="PSUM"))
    pT = actx.enter_context(tc.tile_pool(name="pT", bufs=1, space="PSUM"))

    # static evacuation engine split: Act takes 1x1 evacs (+squares),
    # DVE takes dw evacs (+v stt chains); phC alternates.
    def evac_act(dst, src):
        return nc.scalar.copy(out=dst, in_=src)

    def evac_dve(dst, src):
        return nc.vector.tensor_copy(out=dst, in_=src)

    # ---- constants (sliced out of the 3 packed weight blobs) ----
    w1qk8 = singles.tile([96, 2, 384], F8)
    nc.sync.dma_start(out=w1qk8[:], in_=E["wb8"][0:96, 0:768].rearrange(
        "p (t m) -> p t m", t=2))
    w1va = singles.tile([128, 192], BF16)
    nc.sync.dma_start(out=w1va[:], in_=E["wb16"][:, 0:192])
    w1vb = singles.tile([64, 192], BF16)
    nc.sync.dma_start(out=w1vb[:], in_=E["wb16"][0:64, 192:384])
    dwqk8 = singles.tile([128, 3, 4, 2, 128], F8)
    nc.sync.dma_start(out=dwqk8[:], in_=E["wb8"][:, 768:3840].rearrange(
        "p (i q t m) -> p i q t m", i=3, q=4, t=2))
    wc_qk = singles.tile([128, 3], F32)
    nc.sync.dma_start(out=wc_qk[:], in_=E["wbf"][:, 0:3])
    rdiag3 = singles.tile([128, 6 * 128], BF16)
    nc.sync.dma_start(out=rdiag3[:], in_=E["wb16"][:, 384:1152])
    wtaps3 = singles.tile([128, 3], F32)
    nc.sync.dma_start(out=wtaps3[:], in_=E["wbf"][:, 3:6])
    rdiag4 = singles.tile([64, 6 * 64], BF16)
    nc.sync.dma_start(out=rdiag4[:], in_=E["wb16"][0:64, 1152:1536])
    wtaps4 = singles.tile([64, 3], F32)
    nc.sync.dma_start(out=wtaps4[:], in_=E["wbf"][0:64, 6:9])
    wpTh = singles.tile([HC, HEADS * C], BF16)
    nc.sync.dma_start(out=wpTh[:], in_=E["wb16"][0:HC, 1536:2304])
    temps = singles.tile([HC, HEADS], F32)
    nc.sync.dma_start(out=temps[:], in_=E["wbf"][0:HC, 9:13])
    ident48 = singles.tile([HC, HC], F32)
    i_id = nc.sync.dma_start(out=ident48[:], in_=E["wbf"][0:HC, 13:61])
    terminals.append(i_id)

    vbar_a = vbar_pool.tile([128, N], BF16)
    vbar_b = vbar_pool.tile([64, N], BF16)
    nsq_all = singles.tile([128, 3, NSTRIP], F32, name="nsq_all", tag="nsq_all")

    # G accumulator: [96, 2, 96] f32, head-pairs packed, lives all of phase A
    G = pG.tile([96, 2, HEADS // 2 * HC], F32)

    last_pe = last_act = last_dve = last_pool = None

    # per-strip gram work emitted TWO strips late so its qkT transposes
    # have ~a full strip of slack before PE's in-order stream needs them
    pend_gram = None
    pend_gram_q = []

    # gram computed TRANSPOSED (G'[d, c] = sum_n k[d] q[c]) so phase B needs
    # only one transpose per head: scale by rk on partitions, transpose,
    # then scale by rq*temp during the psum evac.
    def emit_gram(qkT, s):
        nonlocal last_pe
        for r in range(H_STRIP):
            first = (s == 0 and r == 0)
            last = (s == NSTRIP - 1 and r == H_STRIP - 1)
            for hp in range(2):
                last_pe = nc.tensor.matmul(
                    G[:, hp, :], qkT[:, r, C + hp * 96:C + (hp + 1) * 96],
                    qkT[:, r, hp * 96:(hp + 1) * 96],
                    start=first, stop=last, skip_group_check=True)

    # ---------------- phase A: strips ----------------
    for s in range(NSTRIP):
        y0 = s * H_STRIP
        ytop = max(y0 - 1, 0)
        ybot = min(y0 + H_STRIP + 1, HW)  # exclusive
        # halo reuse: strips s>0 copy rows y0-1,y0 from the previous strip's
        # slab bottom; only rows y0+1..ybot-1 are computed fresh.
        if s == 0:
            cbase_x = 0                   # first computed x column
            wrow = 1                      # slab write row
        else:
            cbase_x = (y0 + 1) * 128
            wrow = 2
        cols = ybot * 128 - cbase_x       # computed cols this strip

        # x loads first and dispatched from the (idle) Pool sequencer so they
        # are neither head-of-line blocked behind the previous strip's qkT
        # transposes (SP.SEQ) nor behind this strip's halo copies (Pool.SEQ).
        x8_t = xpool.tile([96, 2, (H_STRIP + 1) * 128], F8, tag="x8")
        xa_t = xpool.tile([128, (H_STRIP + 1) * 128], BF16, tag="xa")
        xb_t = xpool.tile([64, (H_STRIP + 1) * 128], BF16, tag="xb")
        nc.gpsimd.dma_start(out=x8_t[:, :, :cols], in_=E["x8"][:].rearrange(
            "p (t n) -> p t n", t=2)[:, :, cbase_x:ybot * 128])
        nc.gpsimd.dma_start(out=xa_t[:, :cols], in_=E["xa"][:, cbase_x:ybot * 128])
        nc.gpsimd.dma_start(out=xb_t[:, :cols], in_=E["xb"][:, cbase_x:ybot * 128])

        prev_slabs = None if s == 0 else (slab8, slabv3, slabv4)
        slab8 = [slab_pool.tile([128, PADV], F8, name=f"slab8_{i}", tag=f"slab8_{i}")
                 for i in range(3)]
        slabv3 = slab_pool.tile([128, PADV], BF16, name="slabv3", tag="slabv3")
        slabv4 = slab_pool.tile([64, PADV], BF16, name="slabv4", tag="slabv4")
        if s < 2:
            # zero the border columns once per slab buffer (never rewritten)
            for t_ in slab8 + [slabv3, slabv4]:
                nc.vector.memset(_border_ap(t_, 0), 0.0)
                nc.vector.memset(_border_ap(t_, ROWP - 1), 0.0)
        if s == 0:
            for t_ in slab8 + [slabv3, slabv4]:
                nc.vector.memset(t_[:, 1:129], 0.0)  # top halo row
        else:
            po8, pov3, pov4 = prev_slabs
            for dst, srcp in zip(slab8 + [slabv3, slabv4],
                                 po8 + [pov3, pov4]):
                last_pool = nc.gpsimd.tensor_copy(
                    out=dst[:, 0:2 * ROWP],
                    in_=srcp[:, 16 * ROWP:18 * ROWP])

        if s == NSTRIP - 1:
            for t_ in slab8 + [slabv3, slabv4]:
                nc.vector.memset(t_[:, 17 * ROWP + 1:17 * ROWP + 129], 0.0)

        qk_sb = [qk_pool.tile([128, OUTC], BF16, name=f"qk{i}", tag=f"qk{i}")
                 for i in range(3)]

        # ---- tile emitters: A-tiles drain on Act (1024-wide), B on DVE ----
        ATW = 1024  # A-tile width: 2 psum banks, one wide evac
        ntiles = (cols + ATW - 1) // ATW

        def emit_a(ci, t):
            nonlocal last_act
            base = t * ATW
            w = min(ATW, cols - base)
            if w <= 0:
                return
            row0 = wrow + base // 128
            nrows = w // 128
            if ci < 3:
                ps = pQkv.tile([128, ATW], F32, tag="pqkv", name="psa")
                for j in range(0, w, 512):
                    wj = min(512, w - j)
                    nc.tensor.matmul(
                        ps[:, j:j + wj], w1qk8[:, :, ci * 128:(ci + 1) * 128],
                        x8_t[:, :, base + j:base + j + wj],
                        start=True, stop=True, perf_mode=DR)
                last_act = evac_act(
                    _rows_ap(slab8[ci], row0 * ROWP + 1, nrows),
                    ps[:, :w].rearrange("p (r x) -> p r x", x=128))
            else:
                mb_, msz, slab = [(0, 128, slabv3), (128, 64, slabv4)][ci - 3]
                ps = pQkv.tile([msz, ATW], F32, tag="pqkv", name="psv")
                for j in range(0, w, 512):
                    wj = min(512, w - j)
                    nc.tensor.matmul(ps[:, j:j + wj], w1va[:, mb_:mb_ + msz],
                                     xa_t[:, base + j:base + j + wj],
                                     start=True, stop=False)
                    nc.tensor.matmul(ps[:, j:j + wj], w1vb[:, mb_:mb_ + msz],
                                     xb_t[:, base + j:base + j + wj],
                                     start=False, stop=True)
                last_act = evac_act(
                    _rows_ap(slab, row0 * ROWP + 1, nrows),
                    ps[:, :w].rearrange("p (r x) -> p r x", x=128))

        def emit_b(bi, nt):
            nonlocal last_act, last_dve, last_pe
            orow = OROW + nt * 4  # first output row's slab row
            obase = orow * ROWP + 1
            if bi < 3:  # qk dw chunk bi
                i = bi
                ps = pDw.tile([128, 512], F32, tag="pdw", name="psd")
                for p, (dA, dB) in enumerate(QK_PAIRS):
                    rhs = _pair_rows_ap(slab8[i], obase + dA, dB - dA, 4)
                    nc.tensor.matmul(ps[:], dwqk8[:, i, p, :, :], rhs,
                                     start=(p == 0), stop=(p == 3),
                                     perf_mode=DR)
                dst = qk_sb[i][:, nt * 512:(nt + 1) * 512]
                # evac + center tap: dst = w_c * slab8_center + psum (DVE stt)
                last_dve = nc.vector.scalar_tensor_tensor(
                    out=dst.rearrange("p (r x) -> p r x", x=128),
                    in0=_rows_ap(slab8[i], obase, 4),
                    scalar=wc_qk[:, i:i + 1],
                    in1=ps[:].rearrange("p (r x) -> p r x", x=128),
                    op0=ALU.mult, op1=ALU.add)
            else:  # v dw odd taps: bi==3 -> chunk3, bi==4 -> chunk4
                vb = vbar_a if bi == 3 else vbar_b
                rd = rdiag3 if bi == 3 else rdiag4
                csz = 128 if bi == 3 else 64
                slab = slabv3 if bi == 3 else slabv4
                ps = pDw.tile([csz, 512], F32, tag="pdw", name="psd")
                for t, tap in enumerate(V3_PE_TAPS):
                    last_pe = nc.tensor.matmul(
                        ps[:], rd[:, t * csz:(t + 1) * csz],
                        _rows_ap(slab, obase + _tap_delta(tap), 4),
                        start=(t == 0), stop=(t == 5))
                dst = vb[:, y0 * 128 + nt * 512: y0 * 128 + (nt + 1) * 512]
                last_act = evac_act(dst, ps[:])

        # ---- zipper: interleave A (1x1) and B (dw) tile emission ----
        A = [(ci, t) for ci in range(5) for t in range(ntiles)]
        B = [(bi, nt) for bi in range(5) for nt in range(OUTC // 512)]
        DELAY = 3
        ai = bi_ = 0
        while ai < len(A) or bi_ < len(B):
            if ai < len(A):
                emit_a(*A[ai]); ai += 1
            if ai >= DELAY or ai >= len(A):
                if bi_ < len(B) and (bi_ < (ai - DELAY) + 1 or ai >= len(A)):
                    emit_b(*B[bi_]); bi_ += 1
        # emit the oldest pending gram (two strips back) after this strip's
        # dw work so PE (in-order) does not head-of-line block on its
        # transposes.
        if pend_gram_q:
            emit_gram(*pend_gram_q.pop(0))

        # --- norms: sum of squares per channel on Act ---
        sq_scr = vmul_pool.tile([128, OUTC], BF16, tag="vp", name="sq_scr")
        for i in range(3):
            last_act = nc.scalar.activation(
                out=sq_scr[:], in_=qk_sb[i][:], func=AF.Square,
                accum_out=nsq_all[:, i, s:s + 1])

        # --- qk transpose via DMA XBAR into [x, r, ch]; split across the
        # two HWDGE queues (SP, Act) so the transposes overlap ---
        qkT = qkT_pool.tile([128, H_STRIP, 2 * C], BF16, tag="qkT")
        nc.sync.dma_start_transpose(out=qkT[:, :, 0:128], in_=qk_sb[0][:])
        nc.scalar.dma_start_transpose(out=qkT[:, :, 128:256], in_=qk_sb[1][:])
        nc.sync.dma_start_transpose(out=qkT[:, 0:8, 256:384],
                                    in_=qk_sb[2][:, 0:1024])
        nc.scalar.dma_start_transpose(out=qkT[:, 8:16, 256:384],
                                      in_=qk_sb[2][:, 1024:2048])
        if pend_gram is not None:
            pend_gram_q.append(pend_gram)
        pend_gram = (qkT, s)

        # --- v even taps: mul (4x) into scratch, add (2x) in-place to vbar ---
        va_sl = vbar_a[:, y0 * 128: y0 * 128 + OUTC]
        vb_sl = vbar_b[:, y0 * 128: y0 * 128 + OUTC]
        for ci, (slab, wt, sl, csz) in enumerate(
                [(slabv3, wtaps3, va_sl, 128), (slabv4, wtaps4, vb_sl, 64)]):
            for j, dy in enumerate((-1, 0, 1)):
                o = (OROW + dy) * ROWP + 1
                pr = vmul_pool.tile([csz, OUTC], BF16, tag="vp", name="pr")
                last_dve = nc.vector.tensor_scalar_mul(
                    pr[:].rearrange("p (r x) -> p r x", x=128),
                    _rows_ap(slab, o, H_STRIP), wt[:, j:j + 1])
                last_dve = nc.vector.tensor_add(sl, sl, pr[:])

    # tail grams
    for g in pend_gram_q:
        emit_gram(*g)
    pend_gram_q = []
    if pend_gram is not None:
        emit_gram(*pend_gram)
        pend_gram = None

    # ---------------- phase B ----------------
    Gsb = small.tile([96, 2, 96], F32, tag="gsb")
    last_act = nc.scalar.copy(out=Gsb[:], in_=G[:])

    # batched norm chain: one reduce/sqrt/recip over all 3 chunks
    tot = small.tile([128, 3, 1], F32, tag="tot")
    nc.vector.tensor_reduce(out=tot[:], in_=nsq_all[:], axis=AX.X, op=ALU.add)
    rt = small.tile([128, 3, 1], F32, tag="rt")
    nc.scalar.sqrt(out=rt[:], in_=tot[:])
    rr = small.tile([128, 3, 1], F32, tag="rr")
    nc.vector.reciprocal(out=rr[:], in_=rt[:])

    def dmaq(**kw):
        return nc.sync.dma_start(**kw)

    def gather_head(dst, global_base):
        done = 0
        g = global_base
        while done < HC:
            oc, off = g // 128, g % 128
            take = min(HC - done, 128 - off)
            dmaq(out=dst[done:done + take, :],
                 in_=rr[off:off + take, oc, 0:1])
            done += take
            g += take

    mh_sb = []
    for h in range(HEADS):
        hp, off = h // 2, (h % 2) * HC
        rq = small.tile([HC, 1], F32, tag="rq")
        gather_head(rq, h * HC)
        rk = small.tile([HC, 1], F32, tag="rk")
        gather_head(rk, C + h * HC)
        rqt = small.tile([HC, 1], F32, tag="rqt")
        nc.vector.tensor_mul(rqt[:], rq[:], temps[:, h:h + 1])
        g_h = small.tile([HC, HC], F32, tag="gh")
        nc.gpsimd.dma_start(out=g_h[:], in_=Gsb[off:off + HC, hp, off:off + HC])
        # g_h is G'[d, c]; scale rows by rk, transpose, then exp with the
        # rq*temp scale fused in (|z| <= temp, no max-shift needed). The
        # softmax 1/sum normalization commutes through the mh matmul as a
        # per-row scale, applied at the msb evac -- so rs is off the
        # critical path.
        z1 = small.tile([HC, HC], F32, tag="z1")
        nc.vector.tensor_scalar_mul(z1[:], g_h[:], rk[:])
        z1T_ps = pT.tile([HC, HC], F32, tag="ptz")
        nc.tensor.transpose(z1T_ps[:], z1[:], ident48[:])
        e = small.tile([HC, HC], F32, tag="e")
        ssum = small.tile([HC, 1], F32, tag="ssum")
        nc.scalar.activation(out=e[:], in_=z1T_ps[:], func=AF.Exp,
                             scale=rqt[:], accum_out=ssum[:])
        rs = small.tile([HC, 1], F32, tag="rs")
        nc.vector.reciprocal(rs[:], ssum[:])
        attn = small.tile([HC, HC], BF16, tag="attn")
        last_dve = nc.vector.tensor_copy(out=attn[:], in_=e[:])
        mh = pDw.tile([HC, C], F32, tag="pdw")
        nc.tensor.matmul(mh[:], attn[:], wpTh[:, h * C:(h + 1) * C],
                         start=True, stop=True)
        msb = small.tile([HC, C], BF16, tag=f"msb{h}")
        nc.scalar.activation(out=msb[:], in_=mh[:], func=AF.Copy, scale=rs[:])
        mh_sb.append(msb)

    MTa = singles.tile([128, C], BF16)
    MTb = singles.tile([64, C], BF16)
    dmaq(out=MTa[0:48, :], in_=mh_sb[0][:])
    dmaq(out=MTa[48:96, :], in_=mh_sb[1][:])
    dmaq(out=MTa[96:128, :], in_=mh_sb[2][0:32, :])
    dmaq(out=MTb[0:16, :], in_=mh_sb[2][32:48, :])
    i_m = nc.sync.dma_start(out=MTb[16:64, :], in_=mh_sb[3][:])
    terminals.append(i_m)

    # ---------------- phase C: out = blockdiag-attn-proj @ vbar ----------------
    actx.close()  # free phase-A SBUF for wide output staging
    outp = ctx.enter_context(tc.tile_pool(name="outp", bufs=3))
    pC0 = ctx.enter_context(tc.tile_pool(name="pC0", bufs=2, space="PSUM"))
    pC1 = ctx.enter_context(tc.tile_pool(name="pC1", bufs=2, space="PSUM"))
    BLK = 1024
    for blk in range(N // BLK):
        ps0 = pC0.tile([128, BLK], F32, tag="pc0")
        ps1 = pC1.tile([64, BLK], F32, tag="pc1")
        for j in range(BLK // 512):
            sl = slice(blk * BLK + j * 512, blk * BLK + (j + 1) * 512)
            jsl = slice(j * 512, (j + 1) * 512)
            nc.tensor.matmul(ps0[:, jsl], MTa[:, 0:128], vbar_a[:, sl], start=True, stop=False)
            nc.tensor.matmul(ps0[:, jsl], MTb[:, 0:128], vbar_b[:, sl], start=False, stop=True)
            nc.tensor.matmul(ps1[:, jsl], MTa[:, 128:192], vbar_a[:, sl], start=True, stop=False)
            last_pe = nc.tensor.matmul(ps1[:, jsl], MTb[:, 128:192], vbar_b[:, sl],
                                       start=False, stop=True)
        o0 = outp.tile([128, BLK], BF16, tag="o0")
        o1 = outp.tile([64, BLK], BF16, tag="o1")
        if blk % 2 == 0:
            last_act = nc.scalar.copy(out=o0[:], in_=ps0[:])
            last_dve = nc.vector.tensor_copy(out=o1[:], in_=ps1[:])
        else:
            last_dve = nc.vector.tensor_copy(out=o0[:], in_=ps0[:])
            last_act = nc.scalar.copy(out=o1[:], in_=ps1[:])
        i0 = nc.sync.dma_start(out=E["out"][0:128, blk * BLK:(blk + 1) * BLK], in_=o0[:])
        i1 = nc.gpsimd.dma_start(out=E["out"][128:192, blk * BLK:(blk + 1) * BLK], in_=o1[:])
        terminals.append(i0)
        terminals.append(i1)

    terminals.append(last_pe)
    terminals.append(last_act)
    terminals.append(last_dve)
    terminals.append(last_pool)


# ----------------------------------------------------------------------------
# Public entry point: full inputs -> full output, 8-way data-parallel over
# batch across NeuronCores 0-7.
# ----------------------------------------------------------------------------

def kernel(x, w_qkv, w_dw, temperature, w_proj):
    from concourse.bass_utils import run_bass_kernel_spmd

    x = np.asarray(x, np.float32)
    B = x.shape[0]
    assert x.shape == (8, C, HW, HW), x.shape

    nc = bass.Bass()
    build(nc)

    wmaps = prep_weights(w_qkv, w_dw, temperature, w_proj)
    xmaps = prep_x(x)
    in_maps = [{**wmaps, **xm} for xm in xmaps]

    res = run_bass_kernel_spmd(nc, in_maps, core_ids=list(range(8)))
    out = np.stack([np.asarray(res.results[b]["out"]).astype(np.float32)
                    .reshape(C, HW, HW) for b in range(B)])
    return out

